# revision 59
# baseline (speedup 1.0000x reference)
"""Multi-head causal attention (B=4, S=2048, D=1024, H=16, E=64) on 8 TRN2 cores.

Sharding: core c handles batch b = c//2 and head-group g = c%2 (8 heads each).
Each core computes its batch's attention for its 8 heads plus the partial
output projection over its 512 feature columns; the host sums the two
head-group partials per batch and adds the bias.

Datapath is bf16 (weights, x, Q/K/V, exp(scores), out-tiles) with fp32 PSUM
accumulation — same PE rate as fp32r but ~half the energy, which avoids the
package power throttle that clamps the PE clock to 1.2 GHz under sustained
8-core fp32 load.

Structure (per core):
  - s-blocks j of 512 rows; QKV(j+1) and output-proj matmuls are pumped as
    PE fillers between attention chunks of block j so the PE never drains.
  - software-pipelined chunk loop: score(k) is emitted BEFORE attnV(k-1) so
    the score matmul of the next chunk executes while exp(k-1) runs on the
    ACT engine; attnV(k-1) then consumes es(k-1) without stalling the PE.
  - PSUM: one shared 3-slot ring (scores/QKV/proj/bcast, 2 banks each) plus
    a single 2-bank attnV accumulator slot.
  - softmax denominators ride along as a 65th 'ones' column of V; the
    normalize spreads the sums row across partitions by DMA (DVE reciprocal
    is ~16x faster on [128,8] than on a [1,512] row), refines with one
    Newton step, and gathers a bf16 row for the K=1 broadcast matmul.
  - the final block's proj is phased: pr0-2 accumulation overlaps the last
    pair's normalize chain; pr3 matmuls and stores follow.
"""

import math as _m

import numpy as np

B, S, D = 4, 2048, 1024
H, E = 16, 64  # global heads, head dim
HL = 8  # heads per core
P = 128
NPAIR = 4  # head pairs per core
DC = 8  # d chunks of 128
NSB = 4  # s-blocks of 512
NSC = 16  # s-chunks of 128
SCALE = 1.0 / np.sqrt(E)

_CACHE = {}


def _build_nc():
    import concourse.bass as bass  # noqa: F401
    import concourse.mybir as mybir
    import concourse.tile as tile
    from concourse import bacc

    F32 = mybir.dt.float32
    F32R = mybir.dt.float32r
    BF16 = mybir.dt.bfloat16
    AF = mybir.ActivationFunctionType

    nc = bacc.Bacc(None, target_bir_lowering=False)

    x_d = nc.declare_dram_parameter("x", [D, S], BF16, isOutput=False)
    wq_d = nc.declare_dram_parameter("wq", [P, DC, 512], BF16, isOutput=False)
    wk_d = nc.declare_dram_parameter("wk", [P, DC, 512], BF16, isOutput=False)
    wv_d = nc.declare_dram_parameter("wv", [P, DC, 512], BF16, isOutput=False)
    wpt_d = nc.declare_dram_parameter("wpt", [P, NPAIR, D], BF16, isOutput=False)
    trib_d = nc.declare_dram_parameter("trimask_bf", [P, P], BF16, isOutput=False)
    y_d = nc.declare_dram_parameter("y", [S, D], F32, isOutput=True)

    with tile.TileContext(nc) as tc:
        with (
            tc.tile_pool(name="const", bufs=1) as pconst,
            tc.tile_pool(name="ktp", bufs=1) as pkt,
            tc.tile_pool(name="vp", bufs=1) as pv_pool,
            tc.tile_pool(name="wp", bufs=1) as pw,
            tc.tile_pool(name="qtp", bufs=2) as pqt,
            tc.tile_pool(name="xtp", bufs=2) as pxt,
            tc.tile_pool(name="es", bufs=4) as pes,
            tc.tile_pool(name="rst", bufs=2) as prst,
            tc.tile_pool(name="stp", bufs=4) as pst,
            tc.tile_pool(name="otp", bufs=3) as pot,
            tc.tile_pool(name="wptp", bufs=1) as pwpt,
            tc.tile_pool(name="yout", bufs=2) as py,
            tc.tile_pool(name="psum", bufs=3, space="PSUM") as pps,
            tc.tile_pool(name="psav", bufs=1, space="PSUM") as ppsav,
        ):
            trib_t = pconst.tile([P, P], BF16)
            nc.sync.dma_start(trib_t[:], trib_d[:])

            # PE warmup: junk matmuls (trib x trib, no extra DMAs) fill the
            # HAM activity window while x/weight DMAs are in flight, so real
            # matmuls start at the 2.4 GHz clock.
            pwu = pps.tile([P, 2, 512], F32, tag="mm", name="pwu")
            for _ in range(48):
                nc.tensor.matmul(
                    pwu[:, 0, 0:128], trib_t[:], trib_t[:],
                    start=True, stop=True,
                )

            kt_t = pkt.tile([P, NPAIR, S], BF16)  # [e-in-pair, pair, t]
            v_t = pv_pool.tile([P, NSC, HL, 65], BF16)  # [s%128, s//128, h, e|1]
            # ones column of V: tri row-broadcast trick (tri*0 + 1)
            nc.vector.tensor_scalar(
                v_t[:, :, :, 64:65].rearrange("p a b c -> p (a b c)"),
                trib_t[:, 0:128], 0.0, 1.0,
                mybir.AluOpType.mult, mybir.AluOpType.add,
            )
            wq_t = pw.tile([P, DC, 512], BF16)
            wk_t = pw.tile([P, DC, 512], BF16)
            wv_t = pw.tile([P, DC, 512], BF16)
            wpt_t = pwpt.tile([P, NPAIR, D], BF16)

            xt_sl = {}
            qt_sl = {}

            def emit_xt_load(jj):
                xv = x_d[:, jj * 512 : (jj + 1) * 512].rearrange(
                    "(dc p) s -> p dc s", p=P
                )
                nc.sync.dma_start(xt_sl[jj % 2][:], xv[:])

            def qk_step(jj, pr, w_t, dst_sel):
                def go():
                    xt_t = xt_sl[jj % 2]
                    pqs = pps.tile([P, 2, 512], F32, tag="mm", name="pq")
                    pq = pqs[:, 0, :]
                    for dc in range(DC):
                        nc.tensor.matmul(
                            pq[:],
                            w_t[:, dc, pr * P : (pr + 1) * P],
                            xt_t[:, dc, :],
                            start=(dc == 0),
                            stop=(dc == DC - 1),
                        )
                    if dst_sel == "q":
                        nc.vector.tensor_copy(qt_sl[jj % 2][:, pr, :], pq[:])
                    else:
                        nc.vector.tensor_copy(
                            kt_t[:, pr, jj * 512 : (jj + 1) * 512], pq[:]
                        )
                return go

            def v_step(jj, sc):
                def go():
                    xt_t = xt_sl[jj % 2]
                    pvs = pps.tile([P, 2, 512], F32, tag="mm", name="pvv")
                    pvv = pvs[:, 0, :]
                    for dc in range(DC):
                        nc.tensor.matmul(
                            pvv[:],
                            xt_t[:, dc, sc * P : (sc + 1) * P],
                            wv_t[:, dc, :],
                            start=(dc == 0),
                            stop=(dc == DC - 1),
                        )
                    nc.vector.tensor_copy(
                        v_t[:, jj * 4 + sc, :, 0:64],
                        pvv[:].rearrange("p (h e) -> p h e", e=64),
                    )
                return go

            def qkv_steps(jj, skip_xt=False, p0_front=False):
                if not skip_xt:
                    xt_sl[jj % 2] = pxt.tile(
                        [P, DC, 512], BF16, tag="xt", name="xtn"
                    )
                    emit_xt_load(jj)
                qt_sl[jj % 2] = pqt.tile(
                    [P, NPAIR, 512], BF16, tag="qt", name="qtn"
                )
                steps = []
                if p0_front:
                    # exactly what pair 0's chunks need, first
                    steps.append(qk_step(jj, 0, wq_t, "q"))
                    steps.append(qk_step(jj, 0, wk_t, "k"))
                    for sc in range(4):
                        steps.append(v_step(jj, sc))
                    for pr in range(1, NPAIR):
                        steps.append(qk_step(jj, pr, wq_t, "q"))
                        steps.append(qk_step(jj, pr, wk_t, "k"))
                    return steps
                # q first (needed at block start), then k, then v
                for pr in range(NPAIR):
                    steps.append(qk_step(jj, pr, wq_t, "q"))
                for pr in range(NPAIR):
                    steps.append(qk_step(jj, pr, wk_t, "k"))
                for sc in range(4):
                    steps.append(v_step(jj, sc))
                return steps

            def proj_steps(j, ot_t):
                # each step is (j, fn); fn may only run once all 4 of block
                # j's bcasts have executed (ot(j) fully written)
                def mk(sc):
                    def go():
                        s0 = j * 512 + sc * P
                        y_t = py.tile([P, D], F32, tag="y", name="yt")
                        pps_ = pps.tile([P, 2, 512], F32, tag="mm", name="ppn")
                        pp0 = pps_[:, 0, :]
                        pp1 = pps_[:, 1, :]
                        for pr in range(NPAIR):
                            for ib, pp in ((0, pp0), (1, pp1)):
                                nc.tensor.matmul(
                                    pp[:],
                                    ot_t[:, pr, sc * P : (sc + 1) * P],
                                    wpt_t[:, pr, ib * 512 : (ib + 1) * 512],
                                    start=(pr == 0),
                                    stop=(pr == NPAIR - 1),
                                )
                        # copy+store each half as soon as it is ready
                        nc.vector.tensor_copy(y_t[:, 0:512], pp0[:])
                        nc.sync.dma_start(y_d[s0 : s0 + P, 0:512], y_t[:, 0:512])
                        nc.vector.tensor_copy(y_t[:, 512:1024], pp1[:])
                        nc.sync.dma_start(
                            y_d[s0 : s0 + P, 512:1024], y_t[:, 512:1024]
                        )
                    return go
                return [(j, mk(sc)) for sc in range(4)]

            # startup: xt(0) first, then weights in need-order
            xt_sl[0] = pxt.tile([P, DC, 512], BF16, tag="xt", name="xt0")
            emit_xt_load(0)
            nc.sync.dma_start(wq_t[:], wq_d[:])
            nc.sync.dma_start(wk_t[:], wk_d[:])
            nc.sync.dma_start(wv_t[:], wv_d[:])
            nc.sync.dma_start(wpt_t[:], wpt_d[:])
            # block 0 QKV up front (dense)
            for step in qkv_steps(0, skip_xt=True):
                step()

            fill_qkv = []
            fill_proj = []  # proj steps runnable in the current block
            proj_ready = []  # proj steps one block away from running
            proj_wait = []  # proj steps just created (delay-by-2)
            deferred = []  # (queued_unit, normalize) — aged 2 units
            deferred2 = []  # (queued_unit, bcast) — aged 8 units
            AGE1, AGE2 = 1, 8
            unit_no = [0]
            pending_attn = [None]
            pending_pair_end = [None]

            def make_attn(pv, pr, i, es, d0, nchunk):
                def go():
                    for hl in range(2):
                        nc.tensor.matmul(
                            pv[0:65, hl, d0:512],
                            v_t[:, i, 2 * pr + hl, 0:65],
                            es[:, hl, d0:512],
                            start=(i == 0),
                            stop=(i == nchunk - 1),
                        )
                return go

            bcasts_run = [0] * NSB

            def make_pair_end(pv, pr, ot_t, jsrc):
                def go():
                    # drain pv promptly in one copy (frees the accumulator);
                    # defer the rest of the normalize into the next pair's
                    # chunk stream
                    st = pst.tile([65, 2, 512], F32R, tag="st")
                    nc.vector.tensor_copy(st[:], pv[0:65, :, :])
                    st0 = st[:, 0, :]
                    st1 = st[:, 1, :]

                    def normalize():
                        # spread sums across partitions (reciprocal on [128,8]
                        # is ~16x faster than on a [1,512] row), refine with
                        # one Newton step, gather back as a bf16 row for the
                        # broadcast matmul
                        sp = prst.tile([P, 32], F32, tag="sp")
                        spb = prst.tile([P, 8], BF16, tag="spb")
                        d_sl = sp[:, 0:8]
                        r0_sl = sp[:, 8:16]
                        t_sl2 = sp[:, 16:24]
                        nc.sync.dma_start(
                            d_sl[0:64, :], st0[64:65, :].bitcast(F32)
                        )
                        nc.sync.dma_start(
                            d_sl[64:128, :], st1[64:65, :].bitcast(F32)
                        )
                        nc.vector.reciprocal(r0_sl, d_sl)
                        nc.vector.tensor_mul(t_sl2, r0_sl, d_sl)
                        nc.vector.tensor_scalar(
                            t_sl2, t_sl2, -1.0, 2.0,
                            mybir.AluOpType.mult, mybir.AluOpType.add,
                        )
                        with nc.allow_low_precision(
                            reason="bf16 softmax denominators are within "
                            "the kernel's error budget"
                        ):
                            nc.vector.tensor_mul(spb[:], r0_sl, t_sl2)
                        rt = prst.tile([1, 2, 512], BF16, tag="rt", bufs=4)
                        nc.sync.dma_start(
                            rt[0:1, :, :].rearrange("p a b -> p (a b)"),
                            spb[:],
                        )
                        # (bcast_mults appended below with the current unit)

                        def bcast_mults():
                            pbcs = pps.tile(
                                [P, 2, 512], F32, tag="mm", name="pbcn"
                            )
                            pbc0 = pbcs[:, 0, :]
                            pbc1 = pbcs[:, 1, :]
                            # K=1 broadcast matmul (tri row 0 is all-ones)
                            nc.tensor.matmul(
                                pbc0[0:64, :], trib_t[0:1, 0:64],
                                rt[0:1, 0, :], start=True, stop=True,
                            )
                            nc.tensor.matmul(
                                pbc1[0:64, :], trib_t[0:1, 0:64],
                                rt[0:1, 1, :], start=True, stop=True,
                            )
                            nc.vector.tensor_mul(
                                ot_t[0:64, pr, :], st0[0:64, :], pbc0[0:64, :]
                            )
                            st1b = pst.tile([65, 512], BF16, tag="st1b")
                            nc.vector.tensor_mul(
                                st1b[0:64, :], st1[0:64, :], pbc1[0:64, :]
                            )
                            nc.sync.dma_start(
                                ot_t[64:128, pr, :], st1b[0:64, :]
                            )
                            bcasts_run[jsrc] += 1
                        deferred2.append((unit_no[0], bcast_mults))

                    deferred.append((unit_no[0], normalize))
                return go

            for j in range(NSB):
                qt_t = qt_sl[j % 2]
                if j < NSB - 1:
                    fill_qkv.extend(qkv_steps(j + 1))
                # proj fillers delayed by 2 blocks (block 3 absorbs the rest)
                fill_proj.extend(proj_ready)
                proj_ready = proj_wait
                proj_wait = []
                if j == NSB - 1:
                    fill_proj.extend(proj_ready)
                    proj_ready = []
                total_chunks = (4 * j + 4) * NPAIR
                done_chunks = 0
                ot_t = pot.tile([P, NPAIR, 512], BF16, tag="ot", name="otn")
                nchunk = 4 * j + 4
                for pr in range(NPAIR):
                    pv = ppsav.tile([P, 2, 512], F32, tag="av", name="pvn")
                    for i in range(nchunk):
                        t_sl = slice(i * P, (i + 1) * P)
                        delta = i * P - j * 512
                        d0 = max(delta, 0)
                        psc = pps.tile([P, 2, 512], F32, tag="mm", name="pscn")
                        nc.tensor.matmul(
                            psc[:, 0, d0:512],
                            kt_t[0:64, pr, t_sl],
                            qt_t[0:64, pr, d0:512],
                            start=True, stop=True, tile_position=(0, 0),
                        )
                        nc.tensor.matmul(
                            psc[:, 1, d0:512],
                            kt_t[64:128, pr, t_sl],
                            qt_t[64:128, pr, d0:512],
                            start=True, stop=True, tile_position=(64, 0),
                        )
                        es = pes.tile([P, 2, 512], BF16, tag="es", name="esn")
                        nc.scalar.activation(
                            es[:, :, d0:512],
                            psc[:, :, d0:512],
                            AF.Exp,
                            scale=float(SCALE),
                        )
                        if delta >= 0:
                            for hl in range(2):
                                nc.vector.tensor_mul(
                                    es[:, hl, delta : delta + P],
                                    es[:, hl, delta : delta + P],
                                    trib_t[:],
                                )
                        # previous chunk's attnV runs while exp(k) is on ACT
                        if pending_attn[0] is not None:
                            pending_attn[0]()
                        if pending_pair_end[0] is not None:
                            pending_pair_end[0]()
                            pending_pair_end[0] = None
                        while deferred and unit_no[0] - deferred[0][0] >= AGE1:
                            deferred.pop(0)[1]()
                        while (
                            deferred2
                            and unit_no[0] - deferred2[0][0] >= AGE2
                        ):
                            deferred2.pop(0)[1]()
                        pending_attn[0] = make_attn(pv, pr, i, es, d0, nchunk)
                        # pump filler so PE never drains (qkv first: it has
                        # the earliest deadline — next block's scores)
                        done_chunks += 1
                        unit_no[0] += 1
                        rem = total_chunks - done_chunks
                        nf = len(fill_qkv) + len(fill_proj)
                        want = nf if rem == 0 else _m.ceil(
                            nf / ((3 * rem) // 4 + 1)
                        )
                        if done_chunks <= 4:
                            # let the xt(j+1) DMA land before the first qkv
                            # filler, so its matmuls don't head-of-line-block
                            # the score stream
                            want = 0
                        for _ in range(want):
                            if fill_qkv:
                                fill_qkv.pop(0)()
                            elif fill_proj and (
                                bcasts_run[fill_proj[0][0]] == NPAIR
                            ):
                                fill_proj.pop(0)[1]()
                            else:
                                break
                    pending_pair_end[0] = make_pair_end(pv, pr, ot_t, j)
                if j < NSB - 1:
                    proj_wait.extend(proj_steps(j, ot_t))
            # ---- tail: flush the pipeline for the last block ----
            if pending_attn[0] is not None:
                pending_attn[0]()
                pending_attn[0] = None
            if pending_pair_end[0] is not None:
                pending_pair_end[0]()
                pending_pair_end[0] = None
            for _, d in deferred:
                d()
            deferred = []
            for _, fn in fill_proj:
                fn()
            fill_proj = []
            for step in fill_qkv:
                step()
            # phased final proj: pr0-2 accumulation (deps long satisfied)
            # covers the last pair's normalize chain; the pr3 matmuls and
            # stores follow once its ot columns land. sc3 reuses the attnV
            # accumulator banks so the bcast matmul still has a free slot.
            j3 = NSB - 1
            pts = [
                pps.tile([P, 2, 512], F32, tag="mm", name="tpp0"),
                pps.tile([P, 2, 512], F32, tag="mm", name="tpp1"),
                ppsav.tile([P, 2, 512], F32, tag="av", name="tpav"),
            ]
            for sc in range(3):
                for pr in range(3):
                    for ib in range(2):
                        nc.tensor.matmul(
                            pts[sc][:, ib, :],
                            ot_t[:, pr, sc * P : (sc + 1) * P],
                            wpt_t[:, pr, ib * 512 : (ib + 1) * 512],
                            start=(pr == 0), stop=False,
                        )
            for _, d in deferred2:
                d()
            deferred2 = []

            def tail_store(sc, pt):
                y_t = py.tile([P, D], F32, tag="y", name="yt")
                s0 = j3 * 512 + sc * P
                for ib in range(2):
                    nc.tensor.matmul(
                        pt[:, ib, :],
                        ot_t[:, 3, sc * P : (sc + 1) * P],
                        wpt_t[:, 3, ib * 512 : (ib + 1) * 512],
                        start=False, stop=True,
                    )
                nc.vector.tensor_copy(y_t[:, 0:512], pt[:, 0, :])
                nc.sync.dma_start(y_d[s0 : s0 + P, 0:512], y_t[:, 0:512])
                nc.scalar.copy(y_t[:, 512:1024], pt[:, 1, :])
                nc.sync.dma_start(
                    y_d[s0 : s0 + P, 512:1024], y_t[:, 512:1024]
                )

            for sc in range(3):
                tail_store(sc, pts[sc])
            pt3 = pps.tile([P, 2, 512], F32, tag="mm", name="tpp3")
            for pr in range(3):
                for ib in range(2):
                    nc.tensor.matmul(
                        pt3[:, ib, :],
                        ot_t[:, pr, 3 * P : 4 * P],
                        wpt_t[:, pr, ib * 512 : (ib + 1) * 512],
                        start=(pr == 0), stop=False,
                    )
            tail_store(3, pt3)

    nc.compile()
    return nc


def _bf16(a):
    import ml_dtypes

    return np.ascontiguousarray(a).astype(ml_dtypes.bfloat16)


def _host_inputs(Wq, Wk, Wv, Wp):
    """Per-head-group device weight layouts (bf16)."""

    def wdev(W, g):
        # W [16, 1024, 64] -> local [8, D, E] -> [D, 512] -> [P, DC, 512]
        Ws = W[g * HL : (g + 1) * HL]  # [8, D, E]
        A = Ws.transpose(1, 0, 2).reshape(D, HL * E)  # [d, h*64+e]
        return _bf16(A.reshape(DC, P, HL * E).transpose(1, 0, 2))

    def wptdev(Wp, g):
        # Wp [D, D]; j slice -> [512, D] -> [P, NPAIR, D]
        A = Wp[:, g * 512 : (g + 1) * 512].T  # [j, i]
        return _bf16(A.reshape(NPAIR, P, D).transpose(1, 0, 2))

    out = {}
    for g in range(2):
        out[g] = {
            "wq": wdev(Wq, g),
            "wk": wdev(Wk, g),
            "wv": wdev(Wv, g),
            "wpt": wptdev(Wp, g),
        }
    return out


def _consts():
    iot = np.arange(P)
    trimask = (iot[:, None] <= iot[None, :]).astype(np.float32)
    return {
        "trimask_bf": _bf16(trimask),
    }


def make_in_maps(x, Wq, Wk, Wv, Wp):
    """Per-core input dicts (shared by kernel() and the profile harness)."""
    wmaps = _host_inputs(Wq, Wk, Wv, Wp)
    consts = _consts()
    in_maps = []
    for c in range(8):
        b, g = c // 2, c % 2
        m = {"x": _bf16(np.asarray(x)[b].T)}
        m.update(wmaps[g])
        m.update(consts)
        in_maps.append(m)
    return in_maps


def kernel(x, Wq, Wk, Wv, Wp, bp):
    from concourse.bass_utils import run_bass_kernel_spmd

    x = np.asarray(x, dtype=np.float32)
    Wq = np.asarray(Wq, dtype=np.float32)
    Wk = np.asarray(Wk, dtype=np.float32)
    Wv = np.asarray(Wv, dtype=np.float32)
    Wp = np.asarray(Wp, dtype=np.float32)
    bp = np.asarray(bp, dtype=np.float32)

    if "nc" not in _CACHE:
        _CACHE["nc"] = _build_nc()
    nc = _CACHE["nc"]

    in_maps = make_in_maps(x, Wq, Wk, Wv, Wp)
    res = run_bass_kernel_spmd(nc, in_maps, list(range(8)))
    out = np.empty((B, S, D), np.float32)
    for b in range(B):
        out[b] = res.results[2 * b]["y"] + res.results[2 * b + 1]["y"] + bp
    return out
